# revision 25
# baseline (speedup 1.0000x reference)
"""Trainium2 Bass kernel for nn_CholecMetric (segment_reduce).

Per-core (1 clip per NeuronCore, data-parallel over N=8):
  score[h,w] = (sum_p iog_max[p] * Gp[p,h,w]) / (sum_p Gp[p,h,w])
  where iog_max[p] = max_t |Gp_p & Gt_t| / |Gt_t|   (0 where undefined)

Layout: hw = k*512 + c with k in [0,128) on partitions, c in [0,512) free.
  gp sbuf tile [128, 33, 512] bf16 (slot 32 = ones), gt [128, 16, 512] bf16,
  filled by SWDGE cast-DMAs (int32 -> bf16) chunked along c so the matmul
  stream starts early and is paced by the data.
  Intersections: 512 accumulating matmuls psum[16,33] over c
  (lhsT = gt[:, :, c], rhs = gp[:, :, c]); col 32 accumulates gt_area.
  w-chain runs on PE/DVE/ACT (identity matrices fed as inputs).
  cover + 1/cover fully overlap phase A; the weighted sum is split over p
  into three chains (DVE STT | ACT prescale -> GPSIMD add | -> DVE add).
"""

import numpy as np

import concourse.bass as bass
import concourse.bacc as bacc
import concourse.tile as tile
from concourse import mybir
from concourse.bass_utils import run_bass_kernel_spmd

N, P, T, H, W = 8, 32, 16, 256, 256
HW = H * W          # 65536
K, C = 128, 512     # hw = k*C + c
NCORES = 8

# cover split: DVE pair-tree on [0,256), GPSIMD tree on [256,384),
# DVE strided reduces on the late-landing tails
COV_DVE_END = 256
COV_GPS_END = 384
COV_TAILS = ((384, 464), (464, 512))
# num p-split: DVE STT p in [0,NP_DVE); ACT prescale feeds GPSIMD adds for
# p in [NP_DVE,NP_GPS) and DVE adds for p in [NP_GPS,P)
NP_DVE = 14
NP_GPS = 27

F32 = mybir.dt.float32
BF16 = mybir.dt.bfloat16
I32 = mybir.dt.int32
ALU = mybir.AluOpType


def _cover_tree(nc, pool, gp_t, cov_out, c0, c1, eng):
    """Pairwise-sum Gp slots 0..31 (bf16, exact) into cov_out[:, c0:c1] (f32)."""
    w = c1 - c0
    if w == 0:
        return
    lv = gp_t  # level 0: 32 slots
    nslots = P
    lvl = 0
    base = c0
    e = getattr(nc, eng)
    while nslots > 2:
        nxt = pool.tile([K, nslots // 2, w], BF16, tag=f"cov{eng}{lvl}")
        for j in range(nslots // 2):
            if lvl == 0:
                e.tensor_tensor(
                    nxt[:, j, :], lv[:, 2 * j, base:base + w],
                    lv[:, 2 * j + 1, base:base + w], ALU.add)
            else:
                e.tensor_tensor(
                    nxt[:, j, :], lv[:, 2 * j, :], lv[:, 2 * j + 1, :], ALU.add)
        lv = nxt
        base = 0
        nslots //= 2
        lvl += 1
    e.tensor_tensor(cov_out[:, c0:c1], lv[:, 0, :], lv[:, 1, :], ALU.add)


def build():
    nc = bacc.Bacc("TRN2", target_bir_lowering=False, debug=False,
                   num_devices=NCORES)
    gp_d = nc.dram_tensor("gp", [P, HW], I32, kind="ExternalInput")
    gt_d = nc.dram_tensor("gt", [T, HW], I32, kind="ExternalInput")
    id16_d = nc.dram_tensor("id16", [T, T], F32, kind="ExternalInput")
    id32_d = nc.dram_tensor("id32", [P, P], F32, kind="ExternalInput")
    out_d = nc.dram_tensor("score", [HW], F32, kind="ExternalOutput")

    gp_r = gp_d.rearrange("p (k c) -> k p c", c=C)   # [128, 32, 512]
    gt_r = gt_d.rearrange("t (k c) -> k t c", c=C)   # [128, 16, 512]
    out_r = out_d.rearrange("(k c) -> k c", c=C)     # [128, 512]

    with tile.TileContext(nc) as tc:
        with (
            tc.tile_pool(name="data", bufs=1) as data,
            tc.tile_pool(name="work", bufs=1) as work,
            tc.tile_pool(name="small", bufs=1) as small,
            tc.tile_pool(name="psum", bufs=1, space="PSUM") as psum,
        ):
            gp_t = data.tile([K, P + 1, C], BF16, tag="gp")
            gt_t = data.tile([K, T, C], BF16, tag="gt")

            # constants
            id16 = small.tile([T, T], F32, tag="id16")
            id32 = small.tile([P, P], F32, tag="id32")
            ones128 = small.tile([1, K], F32, tag="ones128")
            nc.sync.dma_start(out=id16[:], in_=id16_d[:])
            nc.sync.dma_start(out=id32[:], in_=id32_d[:])
            nc.vector.memset(ones128[:], 1.0)
            # ones slot for gt_area via matmul
            nc.vector.memset(gp_t[:, P, :], 1.0)

            # chunked DMA-cast loads (SWDGE casts int32 -> bf16), interleaved
            # so the matmul stream can start as soon as possible
            def ld_gt(c0, c1):
                nc.gpsimd.dma_start(out=gt_t[:, :, c0:c1], in_=gt_r[:, :, c0:c1])

            def ld_gp(c0, c1):
                nc.gpsimd.dma_start(out=gp_t[:, 0:P, c0:c1], in_=gp_r[:, :, c0:c1])

            ld_gt(0, 64)
            ld_gp(0, 64)
            ld_gt(64, 256)
            ld_gp(64, 128)
            ld_gp(128, 256)
            ld_gt(256, 512)
            ld_gp(256, 384)
            ld_gp(384, 464)
            ld_gp(464, 512)

            # intersections + gt_area: psum_i[t, p(/ones)]
            psum_i = psum.tile([T, P + 1], F32, tag="inters")
            for c in range(C):
                nc.tensor.matmul(
                    psum_i[:], gt_t[:, :, c], gp_t[:, :, c],
                    start=(c == 0), stop=(c == C - 1))

            # cover = sum_p Gp: one strided reduce per chunk on DVE only
            # (keeps GPSIMD quiet in phase A -> no SBUF port contention)
            covm = work.tile([K, C], F32, tag="covm")
            for c0, c1 in ((0, 128), (128, 256), (256, 384), (384, 464),
                           (464, 512)):
                v = gp_t[:, 0:P, c0:c1]
                src = bass.AP(tensor=v.tensor, offset=v.offset,
                              ap=[v.ap[0], v.ap[2], v.ap[1]])
                nc.vector.tensor_reduce(covm[:, c0:c1], src,
                                        mybir.AxisListType.X, ALU.add)
            # rcov = 1/max(cover, 0.5): exact for cover>=1; cover==0 -> num==0
            nc.vector.tensor_scalar_max(covm[:], covm[:], 0.5)
            rcov = work.tile([K, C], F32, tag="rcov")
            nc.vector.reciprocal(rcov[:], covm[:])

            # w-chain on PE/DVE/ACT (GPSIMD stays off the critical path)
            areag = small.tile([T, 1], F32, tag="areag")
            nc.vector.tensor_scalar_max(areag[:], psum_i[:, P:P + 1], 0.5)
            rarea = small.tile([T, 1], F32, tag="rarea")
            nc.vector.reciprocal(rarea[:], areag[:])
            iogs = small.tile([T, P], F32, tag="iogs")
            nc.vector.tensor_scalar_mul(iogs[:], psum_i[:, 0:P], rarea[:, 0:1])
            # transpose iogs -> [P, T], reduce max over t -> iog_max [P, 1]
            psum_tr = psum.tile([P, T], F32, tag="tr")
            nc.tensor.transpose(psum_tr[:], iogs[:], id16[:])
            iomax = small.tile([P, 1], F32, tag="iomax")
            nc.vector.tensor_reduce(iomax[:], psum_tr[:],
                                    mybir.AxisListType.X, ALU.max)
            # w as a row: [1, P] = iomax^T @ id32, then broadcast to 128 rows
            psum_wr = psum.tile([1, P], F32, tag="wr")
            nc.tensor.matmul(psum_wr[:], iomax[:], id32[:])
            w_row = small.tile([1, P], F32, tag="wrow")
            nc.scalar.copy(w_row[:], psum_wr[:])
            psum_wb = psum.tile([K, P], F32, tag="wb")
            nc.tensor.matmul(psum_wb[:], ones128[:], w_row[:])
            w_bc = small.tile([K, P], F32, tag="wbc")
            nc.vector.tensor_copy(w_bc[:], psum_wb[:])

            # num = sum_p w[p] * Gp[p], three full-width chains split over p:
            #   chain1: DVE STT accumulate, p in [0, NP_DVE)
            #   chain2: ACT prescale -> GPSIMD add, p in [NP_DVE, NP_GPS)
            #   chain3: ACT prescale -> DVE add, p in [NP_GPS, P)
            acc = work.tile([K, C], F32, tag="acc")
            accg = work.tile([K, C], F32, tag="accg")
            accv = work.tile([K, C], F32, tag="accv")
            nc.vector.tensor_scalar_mul(acc[:], gp_t[:, 0, :], w_bc[:, 0:1])
            for p in range(1, NP_DVE):
                nc.vector.scalar_tensor_tensor(
                    acc[:], gp_t[:, p, :], w_bc[:, p:p + 1], acc[:],
                    ALU.mult, ALU.add)
            with tc.tile_pool(name="gtmp", bufs=4) as gtmp_pool:
                nc.scalar.mul(accg[:], gp_t[:, NP_DVE, :],
                              w_bc[:, NP_DVE:NP_DVE + 1])
                for p in range(NP_DVE + 1, NP_GPS):
                    gtmp = gtmp_pool.tile([K, C], F32, tag="gtmp")
                    nc.scalar.mul(gtmp[:], gp_t[:, p, :], w_bc[:, p:p + 1])
                    nc.gpsimd.tensor_tensor(accg[:], accg[:], gtmp[:], ALU.add)
                nc.scalar.mul(accv[:], gp_t[:, NP_GPS, :],
                              w_bc[:, NP_GPS:NP_GPS + 1])
                for p in range(NP_GPS + 1, P):
                    vtmp = gtmp_pool.tile([K, C], F32, tag="vtmp")
                    nc.scalar.mul(vtmp[:], gp_t[:, p, :], w_bc[:, p:p + 1])
                    nc.vector.tensor_tensor(accv[:], accv[:], vtmp[:], ALU.add)

            # combine chains and divide
            nc.vector.tensor_tensor(accv[:], accv[:], accg[:], ALU.add)
            nc.vector.tensor_tensor(acc[:], acc[:], accv[:], ALU.add)
            score = work.tile([K, C], F32, tag="score")
            nc.vector.tensor_tensor(score[:], acc[:], rcov[:], ALU.mult)

            nc.sync.dma_start(out=out_r[:], in_=score[:])

    nc.compile()
    return nc


_NC_CACHE = None


def _get_nc():
    global _NC_CACHE
    if _NC_CACHE is None:
        _NC_CACHE = build()
    return _NC_CACHE


def kernel(groups_pred: np.ndarray, groups_true: np.ndarray, trace=False,
           **trace_kwargs) -> np.ndarray:
    nc = _get_nc()
    gp = np.ascontiguousarray(np.asarray(groups_pred, dtype=np.int32)).reshape(N, P, HW)
    gt = np.ascontiguousarray(np.asarray(groups_true, dtype=np.int32)).reshape(N, T, HW)
    id16 = np.eye(T, dtype=np.float32)
    id32 = np.eye(P, dtype=np.float32)
    in_maps = [{"gp": gp[n], "gt": gt[n], "id16": id16, "id32": id32}
               for n in range(N)]
    res = run_bass_kernel_spmd(nc, in_maps, list(range(NCORES)), trace=trace,
                               **trace_kwargs)
    out = np.stack([res.results[n]["score"].reshape(H, W) for n in range(N)])
    if trace:
        kernel.last_results = res
    return out.astype(np.float32)
